# revision 1
# baseline (speedup 1.0000x reference)
"""LEM cell (ODE2) Bass kernel for Trainium2, 8-core data-parallel.

Math (per batch row b):
  ti = x @ W_ih.T + b_ih                  # [B, 4H]
  th = y @ W_hh.T + b_hh                  # [B, 3H]
  tdt = dt @ W_dt.T + b_dt                # [B, 2]
  ms_dt_bar = sig(tdt[:,0]) * sig(ti[:, :H]   + th[:, :H])
  ms_dt     = sig(tdt[:,1]) * sig(ti[:, H:2H] + th[:, H:2H])
  z_new = (1-ms_dt) * z + ms_dt * tanh(ti[:, 3H:] + th[:, 2H:3H])
  y_new = (1-ms_dt_bar) * y + ms_dt_bar * tanh(z_new @ W_z.T + b_z + ti[:, 2H:3H])
  returns (y_new, z_new)

Strategy: shard batch across 8 cores (2048 rows each). On-chip everything is
feature-major ([feature_tile=128 partitions, batch columns free]) so no
on-chip transposes are needed: the host pre-transposes x/y/z and pre-packs
the weights into per-output-tile stationary blocks. The i+h sums and the
i_z + z_new@W_z.T sum are obtained for free by accumulating both GEMMs into
the same PSUM bank. Matmuls run as float32r (fp32 bits, full PE rate;
HW rounds internally, ~1.5e-4 rel per K=128 tile).
"""

import sys

_REPO = "/opt/trn_rl_repo"
if _REPO not in sys.path:
    sys.path.insert(0, _REPO)

from contextlib import ExitStack

import numpy as np

import concourse.bacc as bacc
import concourse.bass as bass
import concourse.tile as tile
from concourse import mybir
from concourse.bass_utils import run_bass_kernel_spmd

P = 128
F32 = mybir.dt.float32
F32R = mybir.dt.float32r
AF = mybir.ActivationFunctionType

N_CORES = 8
NINP = 1024
NHID = 1024
BATCH = 16384

LAST_RESULTS = None  # BassKernelResults of the most recent kernel() call


def build_nc(
    K,            # input feature dim (x)
    H,            # hidden dim (y/z)
    B_shard,      # batch rows per core
    panel,        # batch columns kept resident per pass
    chunk,        # matmul moving-dim size (<=512 fp32)
    wdt00, wdt10,  # W_dt scalars (baked immediates; b_dt rides in biasP)
    mm_dt=F32R,
    w_bufs=5,
    ps_bufs=8,
    xy_bufs=None,
):
    NJT = H // P          # output feature tiles (per H-sized group)
    NKT = K // P          # contraction tiles over x features
    NHT = H // P          # contraction tiles over y/z features
    npan = B_shard // panel
    nch = panel // chunk
    if xy_bufs is None:
        xy_bufs = NKT * nch + 6   # one panel's tiles + cross-panel prefetch

    def f32v(ap):
        """fp32 view of an mm-typed AP for DVE/ACT consumers."""
        return ap.bitcast(F32) if mm_dt != F32 else ap

    nc = bacc.Bacc(trn_type="TRN2", target_bir_lowering=False)

    xT = nc.declare_dram_parameter("xT", [K, B_shard], mm_dt, isOutput=False)
    yT = nc.declare_dram_parameter("yT", [H, B_shard], mm_dt, isOutput=False)
    zT = nc.declare_dram_parameter("zT", [H, B_shard], F32, isOutput=False)
    dtr = nc.declare_dram_parameter("dtr", [1, B_shard], F32, isOutput=False)
    # packed stationary blocks: [jt, kin, (kt_a*P+j | kt_b*P+j)]
    Wd2 = nc.declare_dram_parameter("Wd2", [NJT, P, K + H], mm_dt, isOutput=False)
    Wy = nc.declare_dram_parameter("Wy", [NJT, P, K + H], mm_dt, isOutput=False)
    Wd1 = nc.declare_dram_parameter("Wd1", [NJT, P, K + H], mm_dt, isOutput=False)
    Wg3 = nc.declare_dram_parameter("Wg3", [NJT, P, K + H], mm_dt, isOutput=False)
    # last two columns: row 0 holds b_dt[0], b_dt[1]
    biasP = nc.declare_dram_parameter("biasP", [P, 4 * NJT + 2], F32, isOutput=False)

    y_newT = nc.declare_dram_parameter("y_newT", [H, B_shard], F32, isOutput=True)
    z_newT = nc.declare_dram_parameter("z_newT", [H, B_shard], F32, isOutput=True)

    with tile.TileContext(nc) as tc, ExitStack() as ctx:
        cpool = ctx.enter_context(tc.tile_pool(name="cpool", bufs=1))
        xpool = ctx.enter_context(tc.tile_pool(name="xpool", bufs=xy_bufs))
        ypool = ctx.enter_context(tc.tile_pool(name="ypool", bufs=xy_bufs))
        zpool = ctx.enter_context(tc.tile_pool(name="zpool", bufs=2))
        znpool = ctx.enter_context(tc.tile_pool(name="znpool", bufs=NHT))
        wpool = ctx.enter_context(tc.tile_pool(name="wpool", bufs=w_bufs))
        apool = ctx.enter_context(tc.tile_pool(name="apool", bufs=3))
        dpool = ctx.enter_context(tc.tile_pool(name="dpool", bufs=4))
        opool = ctx.enter_context(tc.tile_pool(name="opool", bufs=2))
        bcpool = ctx.enter_context(tc.tile_pool(name="bcpool", bufs=1))
        rpool = ctx.enter_context(tc.tile_pool(name="rpool", bufs=2))
        pspool = ctx.enter_context(tc.tile_pool(name="pspool", bufs=ps_bufs, space="PSUM"))

        bias_sb = cpool.tile([P, 4 * NJT + 2], F32, name="bias_sb")
        nc.sync.dma_start(bias_sb[:], biasP[:, :])

        def bias_ap(g, jt):
            i = g * NJT + jt
            return bias_sb[:, i : i + 1]

        for p in range(npan):
            b0 = p * panel

            def col(c, n=1):
                return slice(b0 + c * chunk, b0 + (c + n) * chunk)

            # chunked input tiles; cold-start-friendly DMA order:
            # sync: dt, first weights; scalar: x(c0), y(c0), then c1
            dt_sb = rpool.tile([1, panel], F32, name="dt_sb", tag="dtr", bufs=1)
            nc.sync.dma_start(dt_sb[:], dtr[0:1, b0 : b0 + panel])

            # per-batch dt gates first: tiny ACT ops must precede the input
            # DMA flood in the ACT FIFO, else bc gates arrive ~40us late
            sg1 = rpool.tile([1, panel], F32, name="sg1", tag="sg")
            nc.scalar.activation(
                sg1[:], dt_sb[:], AF.Sigmoid,
                bias=bias_sb[0:1, 4 * NJT : 4 * NJT + 1], scale=wdt00,
            )
            sg2 = rpool.tile([1, panel], F32, name="sg2", tag="sg")
            nc.scalar.activation(
                sg2[:], dt_sb[:], AF.Sigmoid,
                bias=bias_sb[0:1, 4 * NJT + 1 : 4 * NJT + 2], scale=wdt10,
            )
            bc1 = bcpool.tile([P, panel], F32, name="bc1", tag="bc1")
            nc.gpsimd.partition_broadcast(bc1[:], sg1[0:1, :])
            bc2 = bcpool.tile([P, panel], F32, name="bc2", tag="bc2")
            nc.gpsimd.partition_broadcast(bc2[:], sg2[0:1, :])

            x_t = [[None] * nch for _ in range(NKT)]
            y_t = [[None] * nch for _ in range(NHT)]

            def load_x(kt, c):
                xt_ = xpool.tile([P, chunk], mm_dt, name="xt", tag="xt")
                nc.scalar.dma_start(xt_[:], xT[kt * P : (kt + 1) * P, col(c)])
                x_t[kt][c] = xt_

            def load_y(kt, c):
                yt_ = ypool.tile([P, chunk], mm_dt, name="yt", tag="yt")
                nc.sync.dma_start(yt_[:], yT[kt * P : (kt + 1) * P, col(c)])
                y_t[kt][c] = yt_

            def load_w(Wsrc, jt, name):
                w_sb = wpool.tile([P, K + H], mm_dt, name=name, tag="w")
                nc.sync.dma_start(w_sb[:, 0:K], Wsrc[jt][:, 0:K])
                nc.scalar.dma_start(w_sb[:, K : K + H], Wsrc[jt][:, K : K + H])
                return w_sb

            def load_w_half(Wsrc, jt, w_sb, half):
                eng = nc.sync if half == 0 else nc.scalar
                lo = 0 if half == 0 else K
                hi = K if half == 0 else K + H
                eng.dma_start(w_sb[:, lo:hi], Wsrc[jt][:, lo:hi])

            # staged cold-start: the ih halves (sync) land before y(c0),
            # the hh halves (scalar) after x(c0); two jt's worth prestaged
            n_pre = min(2, NJT)
            pre_w = []
            for jt in range(n_pre):
                wd2_sb = wpool.tile([P, K + H], mm_dt, name="wd2_sb", tag="w")
                wy_sb = wpool.tile([P, K + H], mm_dt, name="wy_sb", tag="w")
                pre_w.append((wd2_sb, wy_sb))
            for idx in range(max(n_pre, nch)):
                if idx < n_pre:
                    load_w_half(Wd2, idx, pre_w[idx][0], 0)
                    load_w_half(Wy, idx, pre_w[idx][1], 0)
                if idx < nch:
                    for kt in range(NKT):
                        load_x(kt, idx)
                if idx < n_pre:
                    load_w_half(Wd2, idx, pre_w[idx][0], 1)
                    load_w_half(Wy, idx, pre_w[idx][1], 1)
                if idx < nch:
                    for kt in range(NHT):
                        load_y(kt, idx)

            def accum_group(ps, w_sb, rhs_a, rhs_b):
                """16-matmul accumulation: sum_k Wa[k].T@a[k] + Wb[k].T@b[k]."""
                n_a = len(rhs_a)
                for kt in range(n_a):
                    nc.tensor.matmul(
                        ps[:],
                        lhsT=w_sb[:, kt * P : (kt + 1) * P],
                        rhs=rhs_a[kt][:],
                        start=(kt == 0),
                        stop=False,
                    )
                n_b = len(rhs_b)
                for kt in range(n_b):
                    nc.tensor.matmul(
                        ps[:],
                        lhsT=w_sb[:, K + kt * P : K + (kt + 1) * P],
                        rhs=rhs_b[kt][:],
                        start=False,
                        stop=(kt == n_b - 1),
                    )

            # ---- phase B: d2 + y gates -> z_new ----
            zn_t = []
            for jt in range(NJT):
                if jt < n_pre:
                    wd2_sb, wy_sb = pre_w[jt]
                else:
                    wd2_sb = load_w(Wd2, jt, "wd2_sb")
                    wy_sb = load_w(Wy, jt, "wy_sb")
                znr = znpool.tile([P, panel], mm_dt, name="znr", tag="zn")
                zn_t.append(znr)
                for c in range(nch):
                    cs = slice(c * chunk, (c + 1) * chunk)
                    z_sb = zpool.tile([P, chunk], F32, name="z_sb", tag="z")
                    nc.gpsimd.dma_start(z_sb[:], zT[jt * P : (jt + 1) * P, col(c)])

                    ps1 = pspool.tile([P, chunk], F32, name="ps1", tag="ps")
                    accum_group(ps1, wd2_sb, [x_t[k][c] for k in range(NKT)],
                                [y_t[k][c] for k in range(NHT)])
                    s2 = apool.tile([P, chunk], F32, name="s2", tag="act")
                    nc.scalar.activation(s2[:], ps1[:], AF.Sigmoid, bias=bias_ap(0, jt), scale=1.0)

                    ps2 = pspool.tile([P, chunk], F32, name="ps2", tag="ps")
                    accum_group(ps2, wy_sb, [x_t[k][c] for k in range(NKT)],
                                [y_t[k][c] for k in range(NHT)])
                    tz = apool.tile([P, chunk], F32, name="tz", tag="act")
                    nc.scalar.activation(tz[:], ps2[:], AF.Tanh, bias=bias_ap(1, jt), scale=1.0)

                    ms2 = dpool.tile([P, chunk], F32, name="ms2", tag="dve")
                    nc.vector.tensor_mul(ms2[:], s2[:], bc2[:, cs])
                    dlt = dpool.tile([P, chunk], F32, name="dlt", tag="dve")
                    nc.vector.tensor_sub(dlt[:], tz[:], z_sb[:])
                    prd = dpool.tile([P, chunk], F32, name="prd", tag="dve")
                    nc.vector.tensor_mul(prd[:], ms2[:], dlt[:])
                    znc = opool.tile([P, chunk], F32, name="znc", tag="znc")
                    nc.vector.tensor_add(znc[:], prd[:], z_sb[:])
                    nc.sync.dma_start(
                        z_newT[jt * P : (jt + 1) * P, col(c)], znc[:]
                    )
                    # rounding cast into the resident fp32r tile for GEMM3
                    nc.gpsimd.dma_start(znr[:, cs], znc[:])

            # ---- phase C: d1 gate + (i_z + z_new @ W_z.T) -> y_new ----
            for jt in range(NJT):
                wd1_sb = load_w(Wd1, jt, "wd1_sb")
                wg3_sb = load_w(Wg3, jt, "wg3_sb")
                for c in range(nch):
                    cs = slice(c * chunk, (c + 1) * chunk)
                    ps3 = pspool.tile([P, chunk], F32, name="ps3", tag="ps")
                    accum_group(ps3, wd1_sb, [x_t[k][c] for k in range(NKT)],
                                [y_t[k][c] for k in range(NHT)])
                    s1 = apool.tile([P, chunk], F32, name="s1", tag="act")
                    nc.scalar.activation(s1[:], ps3[:], AF.Sigmoid, bias=bias_ap(2, jt), scale=1.0)

                    ps4 = pspool.tile([P, chunk], F32, name="ps4", tag="ps")
                    accum_group(ps4, wg3_sb, [x_t[k][c] for k in range(NKT)],
                                [zn_t[h][:, cs] for h in range(NHT)])
                    u = apool.tile([P, chunk], F32, name="u", tag="act")
                    nc.scalar.activation(u[:], ps4[:], AF.Tanh, bias=bias_ap(3, jt), scale=1.0)

                    # yn = (y - ms1*y) + ms1*u; the first two ops only
                    # need s1, so just two DVE ops trail the final tanh
                    ms1 = dpool.tile([P, chunk], F32, name="ms1", tag="dve")
                    nc.vector.tensor_mul(ms1[:], s1[:], bc1[:, cs])
                    my = dpool.tile([P, chunk], F32, name="my", tag="dve")
                    nc.vector.tensor_mul(my[:], ms1[:], f32v(y_t[jt][c][:]))
                    wyp = dpool.tile([P, chunk], F32, name="wyp", tag="dve")
                    nc.vector.tensor_sub(wyp[:], f32v(y_t[jt][c][:]), my[:])
                    mu = dpool.tile([P, chunk], F32, name="mu", tag="dve")
                    nc.vector.tensor_mul(mu[:], ms1[:], u[:])
                    yn = opool.tile([P, chunk], F32, name="yn", tag="yn")
                    nc.vector.tensor_add(yn[:], wyp[:], mu[:])
                    nc.scalar.dma_start(
                        y_newT[jt * P : (jt + 1) * P, col(c)], yn[:]
                    )

    nc.compile()
    return nc


def _pack_pair(Wa, Wb):
    """[jt, kin, kt*P+j] stationary-block packing of two row-major [out, in]
    weight matrices (lhsT blocks: lhsT[kin, j] = W[jt*P+j, kt*P+kin])."""
    def pack(W):
        O, I = W.shape
        njt, nkt = O // P, I // P
        return (
            W.reshape(njt, P, nkt, P).transpose(0, 3, 2, 1).reshape(njt, P, I)
        )
    A = pack(Wa)
    B = pack(Wb)
    return np.ascontiguousarray(np.concatenate([A, B], axis=2), dtype=np.float32)


def pack_host_inputs(x, y, z, dt, W_ih, b_ih, W_hh, b_hh, W_z, b_z, b_dt, n_cores):
    """Shard batch across cores; pre-transpose activations; pack weights."""
    B, K = x.shape
    H = y.shape[1]
    NJT = H // P
    Bs = B // n_cores

    xT = np.ascontiguousarray(x.T)
    yT = np.ascontiguousarray(y.T)
    zT = np.ascontiguousarray(z.T)
    dtrow = np.ascontiguousarray(dt.reshape(1, B))

    Wd2 = _pack_pair(W_ih[H : 2 * H], W_hh[H : 2 * H])
    Wy = _pack_pair(W_ih[3 * H : 4 * H], W_hh[2 * H : 3 * H])
    Wd1 = _pack_pair(W_ih[0:H], W_hh[0:H])
    Wg3 = _pack_pair(W_ih[2 * H : 3 * H], W_z)

    def bias_cols(bvec):
        return bvec.reshape(NJT, P).T  # [P, NJT]

    bdt_cols = np.zeros((P, 2), np.float32)
    bdt_cols[0, 0] = b_dt[0]
    bdt_cols[0, 1] = b_dt[1]
    biasP = np.ascontiguousarray(
        np.concatenate(
            [
                bias_cols(b_ih[H : 2 * H] + b_hh[H : 2 * H]),
                bias_cols(b_ih[3 * H : 4 * H] + b_hh[2 * H : 3 * H]),
                bias_cols(b_ih[0:H] + b_hh[0:H]),
                bias_cols(b_ih[2 * H : 3 * H] + b_z),
                bdt_cols,
            ],
            axis=1,
        ),
        dtype=np.float32,
    )

    in_maps = []
    for c in range(n_cores):
        cs = slice(c * Bs, (c + 1) * Bs)
        in_maps.append(
            {
                "xT": np.ascontiguousarray(xT[:, cs]),
                "yT": np.ascontiguousarray(yT[:, cs]),
                "zT": np.ascontiguousarray(zT[:, cs]),
                "dtr": np.ascontiguousarray(dtrow[:, cs]),
                "Wd2": Wd2,
                "Wy": Wy,
                "Wd1": Wd1,
                "Wg3": Wg3,
                "biasP": biasP,
            }
        )
    return in_maps


def kernel(x, y, z, dt, W_ih, b_ih, W_hh, b_hh, W_z, b_z, W_dt, b_dt):
    x = np.asarray(x, np.float32)
    y = np.asarray(y, np.float32)
    z = np.asarray(z, np.float32)
    dt = np.asarray(dt, np.float32)
    W_ih = np.asarray(W_ih, np.float32)
    b_ih = np.asarray(b_ih, np.float32)
    W_hh = np.asarray(W_hh, np.float32)
    b_hh = np.asarray(b_hh, np.float32)
    W_z = np.asarray(W_z, np.float32)
    b_z = np.asarray(b_z, np.float32)
    W_dt = np.asarray(W_dt, np.float32)
    b_dt = np.asarray(b_dt, np.float32)

    B, K = x.shape
    H = y.shape[1]
    Bs = B // N_CORES

    in_maps = pack_host_inputs(
        x, y, z, dt, W_ih, b_ih, W_hh, b_hh, W_z, b_z, b_dt, N_CORES
    )
    nc = build_nc(
        K,
        H,
        Bs,
        panel=1024,
        chunk=512,
        wdt00=float(W_dt[0, 0]),
        wdt10=float(W_dt[1, 0]),
    )
    import os

    trace = os.environ.get("LEM_TRACE", "0") == "1"
    tmpdir = os.environ.get("LEM_TMPDIR") or None
    res = run_bass_kernel_spmd(
        nc, in_maps, list(range(N_CORES)), trace=trace, tmpdir=tmpdir
    )
    global LAST_RESULTS
    LAST_RESULTS = res
    y_newT = np.concatenate([r["y_newT"] for r in res.results], axis=1)
    z_newT = np.concatenate([r["z_newT"] for r in res.results], axis=1)
    return (
        np.ascontiguousarray(y_newT.T, dtype=np.float32),
        np.ascontiguousarray(z_newT.T, dtype=np.float32),
    )



# revision 3
# speedup vs baseline: 1.2503x; 1.2503x over previous
"""LEM cell (ODE2) Bass kernel for Trainium2, 8-core data-parallel, fp8.

Math (per batch row b):
  ti = x @ W_ih.T + b_ih                  # [B, 4H]
  th = y @ W_hh.T + b_hh                  # [B, 3H]
  tdt = dt @ W_dt.T + b_dt                # [B, 2]
  ms_dt_bar = sig(tdt[:,0]) * sig(ti[:, :H]   + th[:, :H])
  ms_dt     = sig(tdt[:,1]) * sig(ti[:, H:2H] + th[:, H:2H])
  z_new = (1-ms_dt) * z + ms_dt * tanh(ti[:, 3H:] + th[:, 2H:3H])
  y_new = (1-ms_dt_bar) * y + ms_dt_bar * tanh(z_new @ W_z.T + b_z + ti[:, 2H:3H])
  returns (y_new, z_new)

Strategy: shard batch across 8 cores (2048 rows each); feature-major on-chip
layout ([128-partition feature tile, batch columns free]).  All eight H^2
GEMM-units run as fp8e4m3 DoubleRow matmuls (2x PE rate): host quantizes
x/y and the weight packs (scaled by 32, folded back via the activation
scale=1/32), z_new is cast to fp8 on-chip for the W_z GEMM.  The i+h sums
and i_z + z_new@W_z.T are accumulated in PSUM across both operand groups.
Pointwise runs in bf16 (2x DVE rate); z/y pointwise inputs and both outputs
travel as bf16.  Whole 2048-row shard is one panel; all 32 weight blocks
(64KB/partition) stay resident; each stationary weight block serves 4
chunk-columns back to back to amortize PE weight loads.
"""

import sys

_REPO = "/opt/trn_rl_repo"
if _REPO not in sys.path:
    sys.path.insert(0, _REPO)

from contextlib import ExitStack

import numpy as np
import ml_dtypes

import concourse.bacc as bacc
import concourse.tile as tile
from concourse import mybir
from concourse.bass_utils import run_bass_kernel_spmd

P = 128
F32 = mybir.dt.float32
BF16 = mybir.dt.bfloat16
FP8 = mybir.dt.float8e4
AF = mybir.ActivationFunctionType
PM = mybir.MatmulPerfMode

E4NP = ml_dtypes.float8_e4m3
BFNP = ml_dtypes.bfloat16

N_CORES = 8
NINP = 1024
NHID = 1024
BATCH = 16384
WSCALE = 32.0  # weight pre-scale before fp8 quantization (power of 2)

LAST_RESULTS = None  # BassKernelResults of the most recent kernel() call


def build_nc(
    K,            # input feature dim (x)
    H,            # hidden dim (y/z)
    B_shard,      # batch rows per core
    chunk,        # matmul moving-dim size (psum free size, <=512)
    wdt00, wdt10,  # W_dt scalars (baked immediates; b_dt rides in biasP)
):
    NJT = H // P           # output feature tiles (per H-sized group)
    NKT = K // P           # x contraction 128-blocks
    NHT = H // P           # y/z contraction 128-blocks
    NKP = NKT // 2         # x contraction 256-pairs (DoubleRow)
    NHP = NHT // 2
    nch = B_shard // chunk

    nc = bacc.Bacc(trn_type="TRN2", target_bir_lowering=False)

    x8 = nc.declare_dram_parameter("x8", [K // 2, 2, B_shard], FP8, isOutput=False)
    y8 = nc.declare_dram_parameter("y8", [H // 2, 2, B_shard], FP8, isOutput=False)
    ybf = nc.declare_dram_parameter("ybf", [H, B_shard], BF16, isOutput=False)
    zbf = nc.declare_dram_parameter("zbf", [H, B_shard], BF16, isOutput=False)
    dtr = nc.declare_dram_parameter("dtr", [1, B_shard], F32, isOutput=False)
    # packed stationary blocks: [jt, kin, kidx, j]; kidx 0..NKT-1 = x features,
    # NKT..NKT+NHT-1 = y (or z_new) features; values are fp8(32*W)
    NKH = NKT + NHT
    Wd2 = nc.declare_dram_parameter("Wd2", [NJT, P, NKH, P], FP8, isOutput=False)
    Wy = nc.declare_dram_parameter("Wy", [NJT, P, NKH, P], FP8, isOutput=False)
    Wd1 = nc.declare_dram_parameter("Wd1", [NJT, P, NKH, P], FP8, isOutput=False)
    Wg3 = nc.declare_dram_parameter("Wg3", [NJT, P, NKH, P], FP8, isOutput=False)
    # last two columns: row 0 holds b_dt[0], b_dt[1]
    biasP = nc.declare_dram_parameter("biasP", [P, 4 * NJT + 2], F32, isOutput=False)

    y_newT = nc.declare_dram_parameter("y_newT", [H, B_shard], BF16, isOutput=True)
    z_newT = nc.declare_dram_parameter("z_newT", [H, B_shard], BF16, isOutput=True)

    with tile.TileContext(nc) as tc, ExitStack() as ctx:
        cpool = ctx.enter_context(tc.tile_pool(name="cpool", bufs=1))
        wpool = ctx.enter_context(tc.tile_pool(name="wpool", bufs=1))
        x8pool = ctx.enter_context(tc.tile_pool(name="x8pool", bufs=NKP * nch))
        y8pool = ctx.enter_context(tc.tile_pool(name="y8pool", bufs=NHP * nch))
        zpool = ctx.enter_context(tc.tile_pool(name="zpool", bufs=6))
        ypool = ctx.enter_context(tc.tile_pool(name="ypool", bufs=8))
        bcpool = ctx.enter_context(tc.tile_pool(name="bcpool", bufs=1))
        rpool = ctx.enter_context(tc.tile_pool(name="rpool", bufs=1))
        apool = ctx.enter_context(tc.tile_pool(name="apool", bufs=5))
        dpool = ctx.enter_context(tc.tile_pool(name="dpool", bufs=5))
        opool = ctx.enter_context(tc.tile_pool(name="opool", bufs=3))
        znpool = ctx.enter_context(tc.tile_pool(name="znpool", bufs=1))
        pspool = ctx.enter_context(tc.tile_pool(name="pspool", bufs=8, space="PSUM"))

        bias_sb = cpool.tile([P, 4 * NJT + 2], F32, name="bias_sb")
        nc.sync.dma_start(bias_sb[:], biasP[:, :])

        def bias_ap(g, jt):
            i = g * NJT + jt
            return bias_sb[:, i : i + 1]

        def cs(c):
            return slice(c * chunk, (c + 1) * chunk)

        # per-batch dt gates first: tiny ACT ops must precede the PSUM
        # activations in the ACT FIFO so the bc gates are ready early
        dt_sb = rpool.tile([1, B_shard], F32, name="dt_sb")
        nc.sync.dma_start(dt_sb[:], dtr[0:1, :])
        sg1 = rpool.tile([1, B_shard], BF16, name="sg1")
        nc.scalar.activation(
            sg1[:], dt_sb[:], AF.Sigmoid,
            bias=bias_sb[0:1, 4 * NJT : 4 * NJT + 1], scale=wdt00,
        )
        sg2 = rpool.tile([1, B_shard], BF16, name="sg2")
        nc.scalar.activation(
            sg2[:], dt_sb[:], AF.Sigmoid,
            bias=bias_sb[0:1, 4 * NJT + 1 : 4 * NJT + 2], scale=wdt10,
        )
        bc1 = bcpool.tile([P, B_shard], BF16, name="bc1")
        nc.gpsimd.partition_broadcast(bc1[:], sg1[0:1, :])
        bc2 = bcpool.tile([P, B_shard], BF16, name="bc2")
        nc.gpsimd.partition_broadcast(bc2[:], sg2[0:1, :])

        # ---- resident loads ----
        # weights (sync queue): phase-B blocks jt-interleaved first, then
        # phase-C blocks; first jt's blocks gate the cold start.
        w_d2, w_y, w_d1, w_g3 = [], [], [], []
        for jt in range(NJT):
            wt = wpool.tile([P, NKH, P], FP8, name=f"wd2_{jt}", tag=f"wd2_{jt}")
            nc.sync.dma_start(wt[:], Wd2[jt][:, :, :])
            w_d2.append(wt)
            wt = wpool.tile([P, NKH, P], FP8, name=f"wy_{jt}", tag=f"wy_{jt}")
            nc.sync.dma_start(wt[:], Wy[jt][:, :, :])
            w_y.append(wt)
        # x/y fp8 activations (scalar queue), sweep order g-major
        x_t = [[None] * nch for _ in range(NKP)]
        y_t = [[None] * nch for _ in range(NHP)]
        for g in range(NKP):
            for c in range(nch):
                xt_ = x8pool.tile([P, 2, chunk], FP8, name="xt", tag="xt")
                nc.scalar.dma_start(xt_[:], x8[g * P : (g + 1) * P, :, cs(c)])
                x_t[g][c] = xt_
        for g in range(NHP):
            for c in range(nch):
                yt_ = y8pool.tile([P, 2, chunk], FP8, name="yt", tag="yt")
                nc.scalar.dma_start(yt_[:], y8[g * P : (g + 1) * P, :, cs(c)])
                y_t[g][c] = yt_
        for jt in range(NJT):
            wt = wpool.tile([P, NKH, P], FP8, name=f"wd1_{jt}", tag=f"wd1_{jt}")
            nc.sync.dma_start(wt[:], Wd1[jt][:, :, :])
            w_d1.append(wt)
            wt = wpool.tile([P, NKH, P], FP8, name=f"wg3_{jt}", tag=f"wg3_{jt}")
            nc.sync.dma_start(wt[:], Wg3[jt][:, :, :])
            w_g3.append(wt)

        # fp8 z_new, resident for the W_z GEMM: [kin, hidx, batch]
        zn8 = znpool.tile([P, NHT, B_shard], FP8, name="zn8")

        def accum_group(ps_tiles, w_sb, rhs_a, rhs_b):
            """ps[c] = sum_g Wa[g].T@a[g][c] + Wb[g].T@b[g][c], DoubleRow.

            g-major / c-minor order so each stationary block is loaded once
            per nch moving matmuls."""
            n_a = len(rhs_a)
            n_b = len(rhs_b)
            for g in range(n_a):
                lhsT = w_sb[:, 2 * g : 2 * g + 2, :]
                for c in range(len(ps_tiles)):
                    nc.tensor.matmul(
                        ps_tiles[c][:], lhsT=lhsT, rhs=rhs_a[g][c],
                        start=(g == 0), stop=False, perf_mode=PM.DoubleRow,
                    )
            for g in range(n_b):
                lhsT = w_sb[:, NKT + 2 * g : NKT + 2 * g + 2, :]
                for c in range(len(ps_tiles)):
                    nc.tensor.matmul(
                        ps_tiles[c][:], lhsT=lhsT, rhs=rhs_b[g][c],
                        start=False, stop=(g == n_b - 1), perf_mode=PM.DoubleRow,
                    )

        # ---- phase B: d2 + y gates -> z_new ----
        for jt in range(NJT):
            z_sb = []
            for c in range(nch):
                zt_ = zpool.tile([P, chunk], BF16, name="z_sb", tag="z")
                nc.gpsimd.dma_start(zt_[:], zbf[jt * P : (jt + 1) * P, cs(c)])
                z_sb.append(zt_)

            ps1 = [pspool.tile([P, chunk], F32, name="ps1", tag="ps") for _ in range(nch)]
            accum_group(ps1, w_d2[jt],
                        [[x_t[g][c] for c in range(nch)] for g in range(NKP)],
                        [[y_t[g][c] for c in range(nch)] for g in range(NHP)])
            s2 = []
            for c in range(nch):
                t = apool.tile([P, chunk], BF16, name="s2", tag="sg", bufs=6)
                nc.scalar.activation(t[:], ps1[c][:], AF.Sigmoid,
                                     bias=bias_ap(0, jt), scale=1.0 / WSCALE)
                s2.append(t)
            # gm only needs s2 -> issue before the second matmul sweep drains
            gm = []
            for c in range(nch):
                t = dpool.tile([P, chunk], BF16, name="gm", tag="gm", bufs=6)
                nc.vector.tensor_mul(t[:], s2[c][:], bc2[:, cs(c)])
                gm.append(t)

            ps2 = [pspool.tile([P, chunk], F32, name="ps2", tag="ps") for _ in range(nch)]
            accum_group(ps2, w_y[jt],
                        [[x_t[g][c] for c in range(nch)] for g in range(NKP)],
                        [[y_t[g][c] for c in range(nch)] for g in range(NHP)])
            for c in range(nch):
                tz = apool.tile([P, chunk], BF16, name="tz", tag="th", bufs=3)
                nc.scalar.activation(tz[:], ps2[c][:], AF.Tanh,
                                     bias=bias_ap(1, jt), scale=1.0 / WSCALE)
                d = dpool.tile([P, chunk], BF16, name="d", tag="dm", bufs=3)
                nc.vector.tensor_sub(d[:], tz[:], z_sb[c][:])
                m = dpool.tile([P, chunk], BF16, name="m", tag="mm", bufs=3)
                nc.vector.tensor_mul(m[:], gm[c][:], d[:])
                znc = opool.tile([P, chunk], BF16, name="znc", tag="on", bufs=4)
                nc.vector.tensor_add(znc[:], m[:], z_sb[c][:])
                nc.sync.dma_start(z_newT[jt * P : (jt + 1) * P, cs(c)], znc[:])
                # fp8 cast into the resident zn8 for the W_z GEMM
                nc.gpsimd.tensor_scalar_mul(zn8[:, jt, cs(c)], znc[:], 1.0)

        # ---- phase C: d1 gate + (i_z + z_new @ W_z.T) -> y_new ----
        for jt in range(NJT):
            y_sb = []
            for c in range(nch):
                yt_ = ypool.tile([P, chunk], BF16, name="y_sb", tag="y")
                nc.gpsimd.dma_start(yt_[:], ybf[jt * P : (jt + 1) * P, cs(c)])
                y_sb.append(yt_)

            ps3 = [pspool.tile([P, chunk], F32, name="ps3", tag="ps") for _ in range(nch)]
            accum_group(ps3, w_d1[jt],
                        [[x_t[g][c] for c in range(nch)] for g in range(NKP)],
                        [[y_t[g][c] for c in range(nch)] for g in range(NHP)])
            s1 = []
            for c in range(nch):
                t = apool.tile([P, chunk], BF16, name="s1", tag="sg", bufs=6)
                nc.scalar.activation(t[:], ps3[c][:], AF.Sigmoid,
                                     bias=bias_ap(2, jt), scale=1.0 / WSCALE)
                s1.append(t)
            gm1 = []
            for c in range(nch):
                t = dpool.tile([P, chunk], BF16, name="gm1", tag="gm", bufs=6)
                nc.vector.tensor_mul(t[:], s1[c][:], bc1[:, cs(c)])
                gm1.append(t)

            ps4 = [pspool.tile([P, chunk], F32, name="ps4", tag="ps") for _ in range(nch)]
            accum_group(ps4, w_g3[jt],
                        [[x_t[g][c] for c in range(nch)] for g in range(NKP)],
                        [[zn8[:, 2 * g : 2 * g + 2, cs(c)] for c in range(nch)]
                         for g in range(NHP)])
            for c in range(nch):
                u = apool.tile([P, chunk], BF16, name="u", tag="th", bufs=3)
                nc.scalar.activation(u[:], ps4[c][:], AF.Tanh,
                                     bias=bias_ap(3, jt), scale=1.0 / WSCALE)
                d = dpool.tile([P, chunk], BF16, name="dy", tag="dm", bufs=3)
                nc.vector.tensor_sub(d[:], u[:], y_sb[c][:])
                m = dpool.tile([P, chunk], BF16, name="my", tag="mm", bufs=3)
                nc.vector.tensor_mul(m[:], gm1[c][:], d[:])
                yn = opool.tile([P, chunk], BF16, name="yn", tag="on", bufs=4)
                nc.vector.tensor_add(yn[:], m[:], y_sb[c][:])
                nc.scalar.dma_start(y_newT[jt * P : (jt + 1) * P, cs(c)], yn[:])

    nc.compile()
    return nc


def _pack_pair_fp8(Wa, Wb):
    """[jt, kin, kidx, j] stationary-block packing of two row-major [out, in]
    weight matrices, quantized to fp8(32*W)."""
    def pack(W):
        O, I = W.shape
        njt, nkt = O // P, I // P
        Wq = np.asarray(W * WSCALE, dtype=E4NP)
        # [jt, j, kt, kin] -> [jt, kin, kt, j]
        return Wq.reshape(njt, P, nkt, P).transpose(0, 3, 2, 1)
    return np.ascontiguousarray(np.concatenate([pack(Wa), pack(Wb)], axis=2))


def _pack_act_fp8(aT):
    """[K, B] fp8 -> [K//2, 2, B] DoubleRow pair-major packing."""
    Kdim, B = aT.shape
    nkp = Kdim // (2 * P)
    return np.ascontiguousarray(
        aT.reshape(nkp, 2, P, B).transpose(0, 2, 1, 3).reshape(Kdim // 2, 2, B)
    )


def pack_host_inputs(x, y, z, dt, W_ih, b_ih, W_hh, b_hh, W_z, b_z, b_dt, n_cores):
    """Shard batch across cores; quantize + pre-transpose activations;
    pack weights."""
    B, K = x.shape
    H = y.shape[1]
    NJT = H // P
    Bs = B // n_cores

    x8 = _pack_act_fp8(np.ascontiguousarray(np.asarray(x, dtype=E4NP).T))
    y8 = _pack_act_fp8(np.ascontiguousarray(np.asarray(y, dtype=E4NP).T))
    ybf = np.ascontiguousarray(np.asarray(y, dtype=BFNP).T)
    zbf = np.ascontiguousarray(np.asarray(z, dtype=BFNP).T)
    dtrow = np.ascontiguousarray(dt.reshape(1, B))

    Wd2 = _pack_pair_fp8(W_ih[H : 2 * H], W_hh[H : 2 * H])
    Wy = _pack_pair_fp8(W_ih[3 * H : 4 * H], W_hh[2 * H : 3 * H])
    Wd1 = _pack_pair_fp8(W_ih[0:H], W_hh[0:H])
    Wg3 = _pack_pair_fp8(W_ih[2 * H : 3 * H], W_z)

    def bias_cols(bvec):
        return bvec.reshape(NJT, P).T  # [P, NJT]

    bdt_cols = np.zeros((P, 2), np.float32)
    bdt_cols[0, 0] = b_dt[0]
    bdt_cols[0, 1] = b_dt[1]
    biasP = np.ascontiguousarray(
        np.concatenate(
            [
                bias_cols(b_ih[H : 2 * H] + b_hh[H : 2 * H]),
                bias_cols(b_ih[3 * H : 4 * H] + b_hh[2 * H : 3 * H]),
                bias_cols(b_ih[0:H] + b_hh[0:H]),
                bias_cols(b_ih[2 * H : 3 * H] + b_z),
                bdt_cols,
            ],
            axis=1,
        ),
        dtype=np.float32,
    )

    in_maps = []
    for c in range(n_cores):
        sl = slice(c * Bs, (c + 1) * Bs)
        in_maps.append(
            {
                "x8": np.ascontiguousarray(x8[:, :, sl]),
                "y8": np.ascontiguousarray(y8[:, :, sl]),
                "ybf": np.ascontiguousarray(ybf[:, sl]),
                "zbf": np.ascontiguousarray(zbf[:, sl]),
                "dtr": np.ascontiguousarray(dtrow[:, sl]),
                "Wd2": Wd2,
                "Wy": Wy,
                "Wd1": Wd1,
                "Wg3": Wg3,
                "biasP": biasP,
            }
        )
    return in_maps


def kernel(x, y, z, dt, W_ih, b_ih, W_hh, b_hh, W_z, b_z, W_dt, b_dt):
    x = np.asarray(x, np.float32)
    y = np.asarray(y, np.float32)
    z = np.asarray(z, np.float32)
    dt = np.asarray(dt, np.float32)
    W_ih = np.asarray(W_ih, np.float32)
    b_ih = np.asarray(b_ih, np.float32)
    W_hh = np.asarray(W_hh, np.float32)
    b_hh = np.asarray(b_hh, np.float32)
    W_z = np.asarray(W_z, np.float32)
    b_z = np.asarray(b_z, np.float32)
    W_dt = np.asarray(W_dt, np.float32)
    b_dt = np.asarray(b_dt, np.float32)

    B, K = x.shape
    H = y.shape[1]
    Bs = B // N_CORES

    in_maps = pack_host_inputs(
        x, y, z, dt, W_ih, b_ih, W_hh, b_hh, W_z, b_z, b_dt, N_CORES
    )
    nc = build_nc(
        K,
        H,
        Bs,
        chunk=512,
        wdt00=float(W_dt[0, 0]),
        wdt10=float(W_dt[1, 0]),
    )
    import os

    trace = os.environ.get("LEM_TRACE", "0") == "1"
    tmpdir = os.environ.get("LEM_TMPDIR") or None
    res = run_bass_kernel_spmd(
        nc, in_maps, list(range(N_CORES)), trace=trace, tmpdir=tmpdir
    )
    global LAST_RESULTS
    LAST_RESULTS = res
    y_newT = np.concatenate(
        [np.asarray(r["y_newT"]).astype(np.float32) for r in res.results], axis=1
    )
    z_newT = np.concatenate(
        [np.asarray(r["z_newT"]).astype(np.float32) for r in res.results], axis=1
    )
    return (
        np.ascontiguousarray(y_newT.T, dtype=np.float32),
        np.ascontiguousarray(z_newT.T, dtype=np.float32),
    )


# revision 7
# speedup vs baseline: 2.0502x; 1.6398x over previous
"""LEM cell (ODE2) Bass kernel for Trainium2, 8-core data-parallel, fp8.

Math (per batch row b):
  ti = x @ W_ih.T + b_ih                  # [B, 4H]
  th = y @ W_hh.T + b_hh                  # [B, 3H]
  tdt = dt @ W_dt.T + b_dt                # [B, 2]
  ms_dt_bar = sig(tdt[:,0]) * sig(ti[:, :H]   + th[:, :H])
  ms_dt     = sig(tdt[:,1]) * sig(ti[:, H:2H] + th[:, H:2H])
  z_new = (1-ms_dt) * z + ms_dt * tanh(ti[:, 3H:] + th[:, 2H:3H])
  y_new = (1-ms_dt_bar) * y + ms_dt_bar * tanh(z_new @ W_z.T + b_z + ti[:, 2H:3H])
  returns (y_new, z_new)

Strategy: shard batch across 8 cores (2048 rows each); feature-major on-chip
layout ([128-partition feature tile, batch columns free]).  All eight H^2
GEMM-units run as fp8e4m3 DoubleRow matmuls (2x PE rate): host quantizes
x/y and the weight packs (scaled by 32, folded back via the activation
scale=1/32), z_new is cast to fp8 on-chip for the W_z GEMM.  The i+h sums
and i_z + z_new@W_z.T are accumulated in PSUM across both operand groups.
Pointwise runs in bf16 (2x DVE rate); z/y pointwise inputs and both outputs
travel as bf16.  Whole 2048-row shard is one panel; all 32 weight blocks
(64KB/partition) stay resident; each stationary weight block serves 4
chunk-columns back to back to amortize PE weight loads.
"""

import sys

_REPO = "/opt/trn_rl_repo"
if _REPO not in sys.path:
    sys.path.insert(0, _REPO)

from contextlib import ExitStack

import numpy as np
import ml_dtypes

import concourse.bacc as bacc
import concourse.tile as tile
from concourse import mybir
from concourse.bass_utils import run_bass_kernel_spmd

P = 128
F32 = mybir.dt.float32
BF16 = mybir.dt.bfloat16
FP8 = mybir.dt.float8e4
AF = mybir.ActivationFunctionType
PM = mybir.MatmulPerfMode

E4NP = ml_dtypes.float8_e4m3
BFNP = ml_dtypes.bfloat16

N_CORES = 8
NINP = 1024
NHID = 1024
BATCH = 16384
WSCALE = 32.0  # weight pre-scale before fp8 quantization (power of 2)

LAST_RESULTS = None  # BassKernelResults of the most recent kernel() call


def build_nc(
    K,            # input feature dim (x)
    H,            # hidden dim (y/z)
    B_shard,      # batch rows per core
    chunk,        # matmul moving-dim size (psum free size, <=512)
    wdt00, wdt10,  # W_dt scalars (baked immediates; b_dt rides in biasP)
):
    NJT = H // P           # output feature tiles (per H-sized group)
    NKT = K // P           # x contraction 128-blocks
    NHT = H // P           # y/z contraction 128-blocks
    NKP = NKT // 2         # x contraction 256-pairs (DoubleRow)
    NHP = NHT // 2
    nch = B_shard // chunk

    nc = bacc.Bacc(trn_type="TRN2", target_bir_lowering=False)

    x8 = nc.declare_dram_parameter("x8", [K // 2, 2, B_shard], FP8, isOutput=False)
    y8 = nc.declare_dram_parameter("y8", [H // 2, 2, B_shard], FP8, isOutput=False)
    ybf = nc.declare_dram_parameter("ybf", [H, B_shard], BF16, isOutput=False)
    zbf = nc.declare_dram_parameter("zbf", [H, B_shard], BF16, isOutput=False)
    dtr = nc.declare_dram_parameter("dtr", [1, B_shard], F32, isOutput=False)
    # packed stationary blocks: [jt, kin, kidx, j]; kidx 0..NKT-1 = x features,
    # NKT..NKT+NHT-1 = y (or z_new) features; values are fp8(32*W)
    NKH = NKT + NHT
    Wd2 = nc.declare_dram_parameter("Wd2", [NJT, P, NKH, P], FP8, isOutput=False)
    Wy = nc.declare_dram_parameter("Wy", [NJT, P, NKH, P], FP8, isOutput=False)
    Wd1 = nc.declare_dram_parameter("Wd1", [NJT, P, NKH, P], FP8, isOutput=False)
    Wg3 = nc.declare_dram_parameter("Wg3", [NJT, P, NKH, P], FP8, isOutput=False)
    # last two columns: row 0 holds b_dt[0], b_dt[1]
    biasP = nc.declare_dram_parameter("biasP", [P, 4 * NJT + 2], F32, isOutput=False)

    y_newT = nc.declare_dram_parameter("y_newT", [H, B_shard], F32, isOutput=True)
    z_newT = nc.declare_dram_parameter("z_newT", [H, B_shard], F32, isOutput=True)

    with tile.TileContext(nc) as tc, ExitStack() as ctx:
        dchunk = 2 * chunk          # DVE op width (2 psum chunks)
        ndc = B_shard // dchunk
        cpool = ctx.enter_context(tc.tile_pool(name="cpool", bufs=1))
        wpool = ctx.enter_context(tc.tile_pool(name="wpool", bufs=1))
        x8pool = ctx.enter_context(tc.tile_pool(name="x8pool", bufs=NKP))
        y8pool = ctx.enter_context(tc.tile_pool(name="y8pool", bufs=NHP))
        zpool = ctx.enter_context(tc.tile_pool(name="zpool", bufs=2))
        ypool = ctx.enter_context(tc.tile_pool(name="ypool", bufs=2))
        bcpool = ctx.enter_context(tc.tile_pool(name="bcpool", bufs=1))
        rpool = ctx.enter_context(tc.tile_pool(name="rpool", bufs=1))
        apool = ctx.enter_context(tc.tile_pool(name="apool", bufs=3))
        dpool = ctx.enter_context(tc.tile_pool(name="dpool", bufs=3))
        opool = ctx.enter_context(tc.tile_pool(name="opool", bufs=3))
        znpool = ctx.enter_context(tc.tile_pool(name="znpool", bufs=1))
        pspool = ctx.enter_context(tc.tile_pool(name="pspool", bufs=8, space="PSUM"))

        bias_sb = cpool.tile([P, 4 * NJT + 2], F32, name="bias_sb")
        nc.sync.dma_start(bias_sb[:], biasP[:, :])

        def bias_ap(g, jt):
            i = g * NJT + jt
            return bias_sb[:, i : i + 1]

        def cs(c):
            return slice(c * chunk, (c + 1) * chunk)

        def ds(c2):
            return slice(c2 * dchunk, (c2 + 1) * dchunk)

        # per-batch dt gates first: tiny ACT ops must precede the PSUM
        # activations in the ACT FIFO so the bc gates are ready early
        dt_sb = rpool.tile([1, B_shard], F32, name="dt_sb")
        nc.sync.dma_start(dt_sb[:], dtr[0:1, :])
        sg1 = rpool.tile([1, B_shard], BF16, name="sg1")
        nc.scalar.activation(
            sg1[:], dt_sb[:], AF.Sigmoid,
            bias=bias_sb[0:1, 4 * NJT : 4 * NJT + 1], scale=wdt00,
        )
        sg2 = rpool.tile([1, B_shard], BF16, name="sg2")
        nc.scalar.activation(
            sg2[:], dt_sb[:], AF.Sigmoid,
            bias=bias_sb[0:1, 4 * NJT + 1 : 4 * NJT + 2], scale=wdt10,
        )
        y_t = []
        for g in range(NHP):
            yt_ = y8pool.tile([P, 2, B_shard], FP8, name="yt", tag="yt")
            nc.gpsimd.dma_start(yt_[:], y8[g * P : (g + 1) * P, :, :])
            y_t.append(yt_)
        bc1 = bcpool.tile([P, B_shard], BF16, name="bc1")
        nc.gpsimd.partition_broadcast(bc1[:], sg1[0:1, :])
        bc2 = bcpool.tile([P, B_shard], BF16, name="bc2")
        nc.gpsimd.partition_broadcast(bc2[:], sg2[0:1, :])

        # ---- resident loads ----
        # phase-B weights on sync, x8 on scalar, y8 on vector queue; the
        # first blocks of each gate the cold start.
        w_d2, w_y, w_d1, w_g3 = [], [], [], []
        x_t = []
        for jt in range(NJT):
            wt = wpool.tile([P, NKH, P], FP8, name=f"wd2_{jt}", tag=f"wd2_{jt}")
            nc.sync.dma_start(wt[:], Wd2[jt][:, :, :])
            w_d2.append(wt)
            wt = wpool.tile([P, NKH, P], FP8, name=f"wy_{jt}", tag=f"wy_{jt}")
            nc.sync.dma_start(wt[:], Wy[jt][:, :, :])
            w_y.append(wt)
            if jt < NKP:
                xt_ = x8pool.tile([P, 2, B_shard], FP8, name="xt", tag="xt")
                nc.scalar.dma_start(xt_[:], x8[jt * P : (jt + 1) * P, :, :])
                x_t.append(xt_)
        # phase-C weights stream on gpsimd during phase B
        for jt in range(NJT):
            wt = wpool.tile([P, NKH, P], FP8, name=f"wd1_{jt}", tag=f"wd1_{jt}")
            nc.gpsimd.dma_start(wt[:], Wd1[jt][:, :, :])
            w_d1.append(wt)
            wt = wpool.tile([P, NKH, P], FP8, name=f"wg3_{jt}", tag=f"wg3_{jt}")
            nc.gpsimd.dma_start(wt[:], Wg3[jt][:, :, :])
            w_g3.append(wt)

        # fp8 z_new, resident for the W_z GEMM: [kin, hidx, batch]
        zn8 = znpool.tile([P, NHT, B_shard], FP8, name="zn8")

        def accum_group(ps_tiles, w_sb, rhs_a, rhs_b):
            """ps[c] = sum_g Wa[g].T@a[g][c] + Wb[g].T@b[g][c], DoubleRow.

            g-major / c-minor order so each stationary block is loaded once
            per nch moving matmuls."""
            n_a = len(rhs_a)
            n_b = len(rhs_b)
            for g in range(n_a):
                lhsT = w_sb[:, 2 * g : 2 * g + 2, :]
                for c in range(len(ps_tiles)):
                    nc.tensor.matmul(
                        ps_tiles[c][:], lhsT=lhsT, rhs=rhs_a[g][c],
                        start=(g == 0), stop=False, perf_mode=PM.DoubleRow,
                    )
            for g in range(n_b):
                lhsT = w_sb[:, NKT + 2 * g : NKT + 2 * g + 2, :]
                for c in range(len(ps_tiles)):
                    nc.tensor.matmul(
                        ps_tiles[c][:], lhsT=lhsT, rhs=rhs_b[g][c],
                        start=False, stop=(g == n_b - 1), perf_mode=PM.DoubleRow,
                    )

        def xrhs(g):
            return [x_t[g][:, :, cs(c)] for c in range(nch)]

        def yrhs(g):
            return [y_t[g][:, :, cs(c)] for c in range(nch)]

        # ---- phase B: d2 + y gates -> z_new ----
        for jt in range(NJT):
            jp = slice(jt * P, (jt + 1) * P)
            z16 = zpool.tile([P, B_shard], BF16, name="z16", tag="z")
            nc.gpsimd.dma_start(z16[:], zbf[jp, :])

            ps1 = [pspool.tile([P, chunk], F32, name="ps1", tag="ps") for _ in range(nch)]
            accum_group(ps1, w_d2[jt],
                        [xrhs(g) for g in range(NKP)], [yrhs(g) for g in range(NHP)])
            s2, gm = [], []
            for c2 in range(ndc):
                t = apool.tile([P, dchunk], F32, name="s2", tag="sg", bufs=2)
                nc.scalar.activation(t[:, 0:chunk], ps1[2 * c2][:], AF.Sigmoid,
                                     bias=bias_ap(0, jt), scale=1.0 / WSCALE)
                nc.scalar.activation(t[:, chunk:dchunk], ps1[2 * c2 + 1][:], AF.Sigmoid,
                                     bias=bias_ap(0, jt), scale=1.0 / WSCALE)
                s2.append(t)
                # gm only needs s2 -> issue before the second matmul sweep drains
                g_ = dpool.tile([P, dchunk], F32, name="gm", tag="gm", bufs=2)
                nc.vector.tensor_mul(g_[:], t[:], bc2[:, ds(c2)])
                gm.append(g_)

            ps2 = [pspool.tile([P, chunk], F32, name="ps2", tag="ps") for _ in range(nch)]
            accum_group(ps2, w_y[jt],
                        [xrhs(g) for g in range(NKP)], [yrhs(g) for g in range(NHP)])
            for c2 in range(ndc):
                tz = apool.tile([P, dchunk], F32, name="tz", tag="th", bufs=2)
                nc.scalar.activation(tz[:, 0:chunk], ps2[2 * c2][:], AF.Tanh,
                                     bias=bias_ap(1, jt), scale=1.0 / WSCALE)
                nc.scalar.activation(tz[:, chunk:dchunk], ps2[2 * c2 + 1][:], AF.Tanh,
                                     bias=bias_ap(1, jt), scale=1.0 / WSCALE)
                d = dpool.tile([P, dchunk], F32, name="d", tag="dm", bufs=2)
                nc.vector.tensor_sub(d[:], tz[:], z16[:, ds(c2)])
                m = dpool.tile([P, dchunk], F32, name="m", tag="mm", bufs=2)
                nc.vector.tensor_mul(m[:], gm[c2][:], d[:])
                znc = opool.tile([P, dchunk], F32, name="znc", tag="on")
                nc.vector.tensor_add(znc[:], m[:], z16[:, ds(c2)])
                nc.sync.dma_start(z_newT[jp, ds(c2)], znc[:])
                # fp8 cast into the resident zn8 for the W_z GEMM
                nc.scalar.activation(zn8[:, jt, ds(c2)], znc[:], AF.Copy)

        # ---- phase C: d1 gate + (i_z + z_new @ W_z.T) -> y_new ----
        for jt in range(NJT):
            jp = slice(jt * P, (jt + 1) * P)
            y16 = ypool.tile([P, B_shard], BF16, name="y16", tag="y")
            nc.gpsimd.dma_start(y16[:], ybf[jp, :])

            ps3 = [pspool.tile([P, chunk], F32, name="ps3", tag="ps") for _ in range(nch)]
            accum_group(ps3, w_d1[jt],
                        [xrhs(g) for g in range(NKP)], [yrhs(g) for g in range(NHP)])
            s1, gm1 = [], []
            for c2 in range(ndc):
                t = apool.tile([P, dchunk], F32, name="s1", tag="sg", bufs=2)
                nc.scalar.activation(t[:, 0:chunk], ps3[2 * c2][:], AF.Sigmoid,
                                     bias=bias_ap(2, jt), scale=1.0 / WSCALE)
                nc.scalar.activation(t[:, chunk:dchunk], ps3[2 * c2 + 1][:], AF.Sigmoid,
                                     bias=bias_ap(2, jt), scale=1.0 / WSCALE)
                s1.append(t)
                g_ = dpool.tile([P, dchunk], F32, name="gm1", tag="gm", bufs=2)
                nc.vector.tensor_mul(g_[:], t[:], bc1[:, ds(c2)])
                gm1.append(g_)

            ps4 = [pspool.tile([P, chunk], F32, name="ps4", tag="ps") for _ in range(nch)]
            accum_group(ps4, w_g3[jt],
                        [xrhs(g) for g in range(NKP)],
                        [[zn8[:, 2 * g : 2 * g + 2, cs(c)] for c in range(nch)]
                         for g in range(NHP)])
            for c2 in range(ndc):
                u = apool.tile([P, dchunk], F32, name="u", tag="th", bufs=2)
                nc.scalar.activation(u[:, 0:chunk], ps4[2 * c2][:], AF.Tanh,
                                     bias=bias_ap(3, jt), scale=1.0 / WSCALE)
                nc.scalar.activation(u[:, chunk:dchunk], ps4[2 * c2 + 1][:], AF.Tanh,
                                     bias=bias_ap(3, jt), scale=1.0 / WSCALE)
                d = dpool.tile([P, dchunk], F32, name="dy", tag="dm", bufs=2)
                nc.vector.tensor_sub(d[:], u[:], y16[:, ds(c2)])
                m = dpool.tile([P, dchunk], F32, name="my", tag="mm", bufs=2)
                nc.vector.tensor_mul(m[:], gm1[c2][:], d[:])
                yn = opool.tile([P, dchunk], F32, name="yn", tag="on")
                nc.vector.tensor_add(yn[:], m[:], y16[:, ds(c2)])
                nc.scalar.dma_start(y_newT[jp, ds(c2)], yn[:])

    nc.compile()
    return nc


def _pack_pair_fp8(Wa, Wb):
    """[jt, kin, kidx, j] stationary-block packing of two row-major [out, in]
    weight matrices, quantized to fp8(32*W)."""
    def pack(W):
        O, I = W.shape
        njt, nkt = O // P, I // P
        Wq = np.asarray(W * WSCALE, dtype=E4NP)
        # [jt, j, kt, kin] -> [jt, kin, kt, j]
        return Wq.reshape(njt, P, nkt, P).transpose(0, 3, 2, 1)
    return np.ascontiguousarray(np.concatenate([pack(Wa), pack(Wb)], axis=2))


def _pack_act_fp8(aT):
    """[K, B] fp8 -> [K//2, 2, B] DoubleRow pair-major packing."""
    Kdim, B = aT.shape
    nkp = Kdim // (2 * P)
    return np.ascontiguousarray(
        aT.reshape(nkp, 2, P, B).transpose(0, 2, 1, 3).reshape(Kdim // 2, 2, B)
    )


def pack_host_inputs(x, y, z, dt, W_ih, b_ih, W_hh, b_hh, W_z, b_z, b_dt, n_cores):
    """Shard batch across cores; quantize + pre-transpose activations;
    pack weights."""
    B, K = x.shape
    H = y.shape[1]
    NJT = H // P
    Bs = B // n_cores

    x8 = _pack_act_fp8(np.ascontiguousarray(np.asarray(x, dtype=E4NP).T))
    y8 = _pack_act_fp8(np.ascontiguousarray(np.asarray(y, dtype=E4NP).T))
    ybf = np.ascontiguousarray(np.asarray(y, dtype=BFNP).T)
    zbf = np.ascontiguousarray(np.asarray(z, dtype=BFNP).T)
    dtrow = np.ascontiguousarray(dt.reshape(1, B))

    Wd2 = _pack_pair_fp8(W_ih[H : 2 * H], W_hh[H : 2 * H])
    Wy = _pack_pair_fp8(W_ih[3 * H : 4 * H], W_hh[2 * H : 3 * H])
    Wd1 = _pack_pair_fp8(W_ih[0:H], W_hh[0:H])
    Wg3 = _pack_pair_fp8(W_ih[2 * H : 3 * H], W_z)

    def bias_cols(bvec):
        return bvec.reshape(NJT, P).T  # [P, NJT]

    bdt_cols = np.zeros((P, 2), np.float32)
    bdt_cols[0, 0] = b_dt[0]
    bdt_cols[0, 1] = b_dt[1]
    biasP = np.ascontiguousarray(
        np.concatenate(
            [
                bias_cols(b_ih[H : 2 * H] + b_hh[H : 2 * H]),
                bias_cols(b_ih[3 * H : 4 * H] + b_hh[2 * H : 3 * H]),
                bias_cols(b_ih[0:H] + b_hh[0:H]),
                bias_cols(b_ih[2 * H : 3 * H] + b_z),
                bdt_cols,
            ],
            axis=1,
        ),
        dtype=np.float32,
    )

    in_maps = []
    for c in range(n_cores):
        sl = slice(c * Bs, (c + 1) * Bs)
        in_maps.append(
            {
                "x8": np.ascontiguousarray(x8[:, :, sl]),
                "y8": np.ascontiguousarray(y8[:, :, sl]),
                "ybf": np.ascontiguousarray(ybf[:, sl]),
                "zbf": np.ascontiguousarray(zbf[:, sl]),
                "dtr": np.ascontiguousarray(dtrow[:, sl]),
                "Wd2": Wd2,
                "Wy": Wy,
                "Wd1": Wd1,
                "Wg3": Wg3,
                "biasP": biasP,
            }
        )
    return in_maps


def kernel(x, y, z, dt, W_ih, b_ih, W_hh, b_hh, W_z, b_z, W_dt, b_dt):
    x = np.asarray(x, np.float32)
    y = np.asarray(y, np.float32)
    z = np.asarray(z, np.float32)
    dt = np.asarray(dt, np.float32)
    W_ih = np.asarray(W_ih, np.float32)
    b_ih = np.asarray(b_ih, np.float32)
    W_hh = np.asarray(W_hh, np.float32)
    b_hh = np.asarray(b_hh, np.float32)
    W_z = np.asarray(W_z, np.float32)
    b_z = np.asarray(b_z, np.float32)
    W_dt = np.asarray(W_dt, np.float32)
    b_dt = np.asarray(b_dt, np.float32)

    B, K = x.shape
    H = y.shape[1]
    Bs = B // N_CORES

    in_maps = pack_host_inputs(
        x, y, z, dt, W_ih, b_ih, W_hh, b_hh, W_z, b_z, b_dt, N_CORES
    )
    nc = build_nc(
        K,
        H,
        Bs,
        chunk=512,
        wdt00=float(W_dt[0, 0]),
        wdt10=float(W_dt[1, 0]),
    )
    import os

    trace = os.environ.get("LEM_TRACE", "0") == "1"
    tmpdir = os.environ.get("LEM_TMPDIR") or None
    res = run_bass_kernel_spmd(
        nc, in_maps, list(range(N_CORES)), trace=trace, tmpdir=tmpdir
    )
    global LAST_RESULTS
    LAST_RESULTS = res
    y_newT = np.concatenate([np.asarray(r["y_newT"]) for r in res.results], axis=1)
    z_newT = np.concatenate([np.asarray(r["z_newT"]) for r in res.results], axis=1)
    return (
        np.ascontiguousarray(y_newT.T, dtype=np.float32),
        np.ascontiguousarray(z_newT.T, dtype=np.float32),
    )
